# revision 7
# baseline (speedup 1.0000x reference)
"""BitLinear (RMSNorm + ternary-quantized linear) on 8 trn2 NeuronCores.

Reference math (fp32):
    xn   = x * rsqrt(mean(x^2, -1) + 1e-5) * gamma          # [B,S,K]
    s    = max(mean(|w|), 1e-5)                             # scalar
    q    = round(clip(w / s, -1, 1))                        # ternary {-1,0,1}
    out  = (xn @ q.T) * s                                   # [B,S,Dout]

Identities used by the kernel:
    q = (w > s/2) - (w < -s/2)   (exact, incl. round-half-even at |wn|=0.5)
    out[t,o] = inv[t] * sum_k x[t,k] * (gamma[k]*s*q[o,k])
gamma*s folds into the quantized weight (still exactly +-gamma*s/0 in
fp16) and the epilogue scale is the pure per-token inv. The contraction
runs on the PE in fp16 with fp32 PSUM accumulation.

The ternary pattern is exquisitely sensitive to s: one weight flipped
across the s/2 threshold costs ~1.6e-2 of the 2e-2 rel-err budget
(max-err metric). So the comparisons use f32 w against the exact
full-weight |w|-mean, reduced with the same instruction chain on the
same slices as the validated baseline (w_red 1/8 per core + AllReduce).

x ships as fp16 (host cast; noise-like 2.4e-4, no pattern risk), and is
loaded twice: once [tok,k] for RMSNorm stats (ACT square+accum), once
via xbar DMA-transpose directly from DRAM into xT[p, t, tok] with
k = t*128 + p. All x-side prep is s-independent and fills the
barrier+AllReduce window; the post-s quantize splits across DVE
(pos, add) and GpSimd (neg) and the first two matmul waves are emitted
k-quarter-interleaved across 8 PSUM banks so the PE ramps while the
quantize stream trails in.

Sharding: 2 token-groups x 4 dout-groups (core = rg*4 + cg).
"""

import numpy as np

import concourse.bass as bass
import concourse.tile as tile
from concourse import bacc, mybir
from concourse.bass_utils import run_bass_kernel_spmd

F32 = mybir.dt.float32
F16 = mybir.dt.float16
BF16 = mybir.dt.bfloat16

# Full-problem constants
B, S, K, DOUT = 4, 2048, 2048, 8192
N_CORES = 8
RG, CG = 2, 4  # token groups x dout groups
TOK_SH = (B * S) // RG      # 4096 tokens per core
DOUT_SH = DOUT // CG        # 2048 out-features per core
RED_ROWS = DOUT // N_CORES  # 1024 rows of w reduced per core for mean(|w|)
W_COUNT = float(DOUT * K)
EPS = 1e-5


def build_nc(tok_sh=TOK_SH, k=K, dout_sh=DOUT_SH, n_cores=N_CORES,
             strip_blk=4, use_cc=True):
    """Build the SPMD Bass program (one program, per-core inputs differ)."""
    kt = k // 128            # contraction tiles
    n_strip = tok_sh // 128  # token strips
    n_wtile = dout_sh // 128
    n_blk = n_strip // strip_blk
    n_kq = max(1, kt // 4)
    kq = kt // n_kq
    n_rtile = RED_ROWS // 128
    w_res = 6                # f32 w tiles resident before s lands

    nc = bacc.Bacc("TRN2", target_bir_lowering=False, num_devices=n_cores)

    x_d = nc.declare_dram_parameter("x_sh", [tok_sh, k], F16, isOutput=False)
    w_d = nc.declare_dram_parameter("w_shT", [k, dout_sh], F32, isOutput=False)
    wr_d = nc.declare_dram_parameter("w_red", [RED_ROWS, k], F32,
                                     isOutput=False)
    g_d = nc.declare_dram_parameter("gamma", [k], F32, isOutput=False)
    out_d = nc.declare_dram_parameter("out_sh", [tok_sh, dout_sh], F32,
                                      isOutput=True)

    with tile.TileContext(nc, num_cores=n_cores) as tc:
        with (
            tc.tile_pool(name="consts", bufs=1) as consts,
            tc.tile_pool(name="f32s", bufs=1) as f32s,
            tc.tile_pool(name="f16s", bufs=1) as f16s,
            tc.tile_pool(name="qt", bufs=1) as qtp,
            tc.tile_pool(name="outp", bufs=1) as outp,
            tc.tile_pool(name="psum", bufs=8, space="PSUM") as psum,
            tc.tile_pool(name="dram", bufs=1, space="DRAM") as dram,
        ):
            # ---- constants -------------------------------------------------
            # cblock cols: 0 ones, 1 eps, 2 prev(|w| partial), 3 allv, 4.. parts
            cblock = consts.tile([128, 4 + n_rtile], F32)
            ones_col = cblock[:, 0:1]
            eps_t = cblock[:, 1:2]
            prev = cblock[:, 2:3]
            allv = cblock[:, 3:4]
            parts = cblock[:, 4:4 + n_rtile]
            nc.vector.memset(ones_col, 1.0)
            nc.vector.memset(eps_t, EPS)
            ones_row = consts.tile([1, 128], F32)
            nc.vector.memset(ones_row, 1.0)
            # gamma transposed to [p, t] with k = t*128 + p
            gT = consts.tile([128, kt], F32)
            g_ap = bass.AP(tensor=g_d, offset=0, ap=[[1, 128], [128, kt]])
            nc.sync.dma_start(out=gT, in_=g_ap)
            # sblock cols: 0 s_mean, 1 s_clip, 2 s_bc, 3 t_bc, 4 nt_bc
            sblock = consts.tile([128, 5], F32)
            s_bc = sblock[:, 2:3]
            t_bc = sblock[:, 3:4]
            nt_bc = sblock[:, 4:5]
            gs = consts.tile([128, kt], F32)    # gamma * s
            ngs = consts.tile([128, kt], F32)   # -gamma * s
            invb = consts.tile([128, n_strip], F32)  # per-strip 1/rms

            # ---- phase S-pre: |w| partials + AllReduce (exact mean) --------
            # Same slices + same reduce chain as the validated baseline so the
            # resulting s is bit-identical (ternary pattern must not flip).
            for i in range(n_rtile):
                wt = f32s.tile([128, k], F32, tag="wred", bufs=2,
                               name=f"wr{i}")
                nc.gpsimd.dma_start(out=wt, in_=wr_d[i * 128:(i + 1) * 128, :])
                nc.vector.tensor_reduce(
                    parts[:, i:i + 1], wt, axis=mybir.AxisListType.X,
                    op=mybir.AluOpType.add, apply_absolute_value=True)
            nc.vector.tensor_reduce(prev, parts, axis=mybir.AxisListType.X,
                                    op=mybir.AluOpType.add)
            cc_in = dram.tile([128, 1], F32)
            cc_out = dram.tile([128, 1], F32, addr_space="Shared")
            nc.gpsimd.dma_start(out=cc_in, in_=prev)
            if use_cc:
                nc.gpsimd.collective_compute(
                    "AllReduce", mybir.AluOpType.add,
                    replica_groups=[list(range(n_cores))],
                    ins=[cc_in.opt()], outs=[cc_out.opt()],
                )
            else:
                nc.gpsimd.dma_start(out=cc_out, in_=cc_in)

            # ---- phase W-pre: f32 w tiles resident before s ----------------
            wts = {}
            for i in range(w_res):
                wts[i] = f32s.tile([128, dout_sh], F32, tag="wt", bufs=w_res,
                                   name=f"wq{i}")
                nc.gpsimd.dma_start(out=wts[i],
                                    in_=w_d[i * 128:(i + 1) * 128, :])

            # ---- strip preps (all s-independent; fill the AR window) -------
            xT_tiles = {}

            def prep_strip(j):
                xs = f16s.tile([128, k], F16, tag="xs", bufs=3, name=f"xs{j}")
                nc.gpsimd.dma_start(out=xs, in_=x_d[j * 128:(j + 1) * 128, :])
                xsq = f16s.tile([128, k], BF16, tag="xsq", bufs=1,
                                name=f"xsq{j}")
                sc = f16s.tile([128, 2], F32, tag="sc", bufs=3, name=f"sc{j}")
                ssq, rms = sc[:, 0:1], sc[:, 1:2]
                nc.scalar.activation(xsq, xs,
                                     mybir.ActivationFunctionType.Square,
                                     accum_out=ssq)
                nc.scalar.activation(rms, ssq,
                                     mybir.ActivationFunctionType.Sqrt,
                                     bias=eps_t, scale=1.0 / k)
                nc.vector.reciprocal(invb[:, j:j + 1], rms)
                xT = f16s.tile([128, kt, 128], F16, tag="xT", bufs=8,
                               name=f"xT{j}")
                nc.sync.dma_start_transpose(out=xT,
                                            in_=x_d[j * 128:(j + 1) * 128, :])
                xT_tiles[j] = xT

            for j in range(n_strip):
                prep_strip(j)

            # ---- phase S-post: finish s after the AllReduce ----------------
            # (on gpsimd so the sync-queue transposes can't delay it)
            nc.gpsimd.dma_start(out=allv, in_=cc_out)
            tot_ps = psum.tile([1, 1], F32, tag="mm")
            nc.tensor.matmul(tot_ps, lhsT=allv, rhs=ones_col,
                             start=True, stop=True)
            nc.scalar.activation(sblock[0:1, 0:1], tot_ps,
                                 mybir.ActivationFunctionType.Copy,
                                 scale=1.0 / W_COUNT)
            nc.vector.tensor_scalar_max(sblock[0:1, 1:2], sblock[0:1, 0:1],
                                        EPS)
            s_bc_ps = psum.tile([128, 1], F32, tag="mm")
            nc.tensor.matmul(s_bc_ps, lhsT=ones_row, rhs=sblock[0:1, 1:2],
                             start=True, stop=True)
            nc.scalar.copy(s_bc, s_bc_ps)
            nc.scalar.mul(t_bc, s_bc, 0.5)
            nc.scalar.mul(nt_bc, s_bc, -0.5)
            nc.vector.tensor_scalar(gs, gT, s_bc, None, mybir.AluOpType.mult)
            nc.vector.tensor_scalar(ngs, gs, -1.0, None, mybir.AluOpType.mult)

            # ---- quantize: pos/add on DVE, neg on GpSimd -------------------
            # qQ[q][p, u, o] = gamma*s*q(w[o, (kq*q+u)*128+p]) in fp16
            qQs = [qtp.tile([128, kq, dout_sh], F16, tag=f"qQ{q}",
                            name=f"qQ{q}") for q in range(n_kq)]
            for i in range(kt):
                if i not in wts:
                    wts[i] = f32s.tile([128, dout_sh], F32, tag="wt",
                                       bufs=w_res, name=f"wq{i}")
                    nc.gpsimd.dma_start(out=wts[i],
                                        in_=w_d[i * 128:(i + 1) * 128, :])
                wt = wts[i]
                pos = f16s.tile([128, dout_sh], F16, tag="pos", bufs=2,
                                name=f"pos{i}")
                nc.vector.tensor_scalar(pos, wt, t_bc, gs[:, i:i + 1],
                                        mybir.AluOpType.is_gt,
                                        mybir.AluOpType.mult)
                nm = f16s.tile([128, dout_sh], F16, tag="nm", bufs=2,
                               name=f"nm{i}")
                nc.gpsimd.tensor_scalar(nm, wt, nt_bc, ngs[:, i:i + 1],
                                        mybir.AluOpType.is_lt,
                                        mybir.AluOpType.mult)
                nc.vector.tensor_tensor(qQs[i // kq][:, i % kq, :], pos, nm,
                                        mybir.AluOpType.add)

            # ---- main loop -------------------------------------------------
            def emit_psum_tile(j, d, chunked):
                """One [128,512] output tile; k-chain in 4-MM chunks."""
                ps = psum.tile([128, 512], F32, tag="mm", name=f"ps{j}_{d}")
                if not chunked:
                    for t in range(kt):
                        nc.tensor.matmul(
                            ps, lhsT=xT_tiles[j][:, t, :],
                            rhs=qQs[t // kq][:, t % kq, d * 512:(d + 1) * 512],
                            start=(t == 0), stop=(t == kt - 1))
                    finish_psum_tile(j, d, ps)
                return ps

            def chain_chunk(j, d, ps, qtr):
                for u in range(kq):
                    t = qtr * kq + u
                    nc.tensor.matmul(
                        ps, lhsT=xT_tiles[j][:, t, :],
                        rhs=qQs[qtr][:, u, d * 512:(d + 1) * 512],
                        start=(t == 0), stop=(t == kt - 1))

            def finish_psum_tile(j, d, ps):
                ob = outp.tile([128, 512], F32, tag="ob", bufs=4,
                               name=f"ob{j}_{d}")
                nc.scalar.activation(
                    out=ob, in_=ps,
                    func=mybir.ActivationFunctionType.Copy,
                    scale=invb[:, j:j + 1])
                nc.scalar.dma_start(
                    out=out_d[j * 128:(j + 1) * 128, d * 512:(d + 1) * 512],
                    in_=ob)

            # first two waves: quarter-interleaved across all 8 banks so the
            # PE ramps while quantize quarters trail in
            for wave in range(2):
                tiles = [(j, d) for d in (2 * wave, 2 * wave + 1)
                         for j in range(strip_blk)]
                pss = {}
                for (j, d) in tiles:
                    pss[(j, d)] = emit_psum_tile(j, d, chunked=True)
                for qtr in range(n_kq):
                    for (j, d) in tiles:
                        chain_chunk(j, d, pss[(j, d)], qtr)
                for (j, d) in tiles:
                    finish_psum_tile(j, d, pss[(j, d)])

            # rest: plain deep-pipelined chains (block 0 fully covered by the
            # interleaved waves above)
            for b in range(1, n_blk):
                for d in range(n_wtile // 4):
                    for j in range(b * strip_blk, (b + 1) * strip_blk):
                        emit_psum_tile(j, d, chunked=False)

    nc.compile()
    return nc


_NC_CACHE = {}


def _get_nc():
    if "nc" not in _NC_CACHE:
        _NC_CACHE["nc"] = build_nc()
    return _NC_CACHE["nc"]


def make_in_maps(x, weight, gamma):
    """Shard + lay out host-side. x:[B,S,K] f32, weight:[DOUT,K] f32."""
    x = np.asarray(x, dtype=np.float32)
    weight = np.ascontiguousarray(np.asarray(weight, dtype=np.float32))
    gamma = np.ascontiguousarray(np.asarray(gamma, dtype=np.float32))

    x16 = x.reshape(B * S, K).astype(np.float16)
    wT = np.ascontiguousarray(weight.T)  # [K, DOUT] f32
    in_maps = []
    for c in range(N_CORES):
        rg, cg = c // CG, c % CG
        in_maps.append({
            "x_sh": np.ascontiguousarray(
                x16[rg * TOK_SH:(rg + 1) * TOK_SH]),
            "w_shT": np.ascontiguousarray(
                wT[:, cg * DOUT_SH:(cg + 1) * DOUT_SH]),
            "w_red": weight[c * RED_ROWS:(c + 1) * RED_ROWS],
            "gamma": gamma,
        })
    return in_maps


def kernel(x, weight, gamma):
    in_maps = make_in_maps(x, weight, gamma)
    nc = _get_nc()
    res = run_bass_kernel_spmd(nc, in_maps, list(range(N_CORES))).results

    out = np.empty((B * S, DOUT), dtype=np.float32)
    for c in range(N_CORES):
        rg, cg = c // CG, c % CG
        out[rg * TOK_SH:(rg + 1) * TOK_SH,
            cg * DOUT_SH:(cg + 1) * DOUT_SH] = res[c]["out_sh"]
    return out.reshape(B, S, DOUT)


# revision 11
# speedup vs baseline: 1.6607x; 1.6607x over previous
"""BitLinear (RMSNorm + ternary-quantized linear) on 8 trn2 NeuronCores.

Reference math (fp32):
    xn   = x * rsqrt(mean(x^2, -1) + 1e-5) * gamma          # [B,S,K]
    s    = max(mean(|w|), 1e-5)                             # scalar
    q    = round(clip(w / s, -1, 1))                        # ternary {-1,0,1}
    out  = (xn @ q.T) * s                                   # [B,S,Dout]

Identities used by the kernel:
    q = (w > s/2) - (w < -s/2)   (exact, incl. round-half-even at |wn|=0.5)
    2q = Sign(w - s/2) + Sign(w + s/2)          (same pattern, ACT-friendly)
    out[t,o] = inv[t] * sum_k x[t,k] * (gamma[k]*s*q[o,k])
gamma*s folds into the quantized weight (still exactly +-gamma*s/0 in
fp16) and the epilogue scale is the pure per-token inv. The contraction
runs on the PE in fp16 with fp32 PSUM accumulation.

The ternary pattern is exquisitely sensitive to s: one weight flipped
across the s/2 threshold costs ~1.6e-2 of the 2e-2 rel-err budget
(max-err metric). So the comparisons use f32 w against the exact
full-weight |w|-mean, reduced with the same instruction chain on the
same slices as the validated baseline (w_red 1/8 per core + AllReduce).

x ships as fp16 (host cast; noise-like 2.4e-4, no pattern risk): loaded
once [tok,k] for RMSNorm stats (ACT square+accum) and once via xbar
DMA-transpose directly from DRAM into xT[p, t, tok] with k = t*128 + p.
All x-side prep is s-independent.

Scheduling around the ~100us barrier+AllReduce:
  - the post-AR `allv` copy sits on the gpsimd DMA queue, so every DMA
    emitted after it (w k-tiles 8-15, late x strips) is held back until
    the collective completes - keeps the SDMA engines quiet during the
    AR and overlaps that traffic with the matmul stream instead.
  - post-s quantize splits across DVE (is_gt/is_lt + add, tiles 0-7,
    f32 w resident) and ACT (Sign-pair, tiles 8-15, w streamed), so the
    full q lands ~45us after s instead of ~70.
  - block 0 is emitted k-quarter-interleaved across all 8 PSUM banks so
    the PE ramps while quantize quarters trail in.

Sharding: 2 token-groups x 4 dout-groups (core = rg*4 + cg).
"""

import numpy as np

import concourse.bass as bass
import concourse.tile as tile
from concourse import bacc, mybir
from concourse.bass_utils import run_bass_kernel_spmd

F32 = mybir.dt.float32
F16 = mybir.dt.float16
BF16 = mybir.dt.bfloat16

# Full-problem constants
B, S, K, DOUT = 4, 2048, 2048, 8192
N_CORES = 8
RG, CG = 2, 4  # token groups x dout groups
TOK_SH = (B * S) // RG      # 4096 tokens per core
DOUT_SH = DOUT // CG        # 2048 out-features per core
RED_ROWS = DOUT // N_CORES  # 1024 rows of w reduced per core for mean(|w|)
W_COUNT = float(DOUT * K)
EPS = 1e-5


def build_nc(tok_sh=TOK_SH, k=K, dout_sh=DOUT_SH, n_cores=N_CORES,
             strip_blk=4, use_cc=True):
    """Build the SPMD Bass program (one program, per-core inputs differ)."""
    kt = k // 128            # contraction tiles
    n_strip = tok_sh // 128  # token strips
    n_wtile = dout_sh // 128
    n_blk = n_strip // strip_blk
    n_kq = max(1, kt // 4)
    kq = kt // n_kq
    n_rtile = RED_ROWS // 128
    act_tiles = kt // 2      # k-tiles quantized via the ACT Sign-pair path
    pre_strips = 10          # strips fully prepped before the main loop

    nc = bacc.Bacc("TRN2", target_bir_lowering=False, num_devices=n_cores)

    x_d = nc.declare_dram_parameter("x_sh", [tok_sh, k], F16, isOutput=False)
    w_d = nc.declare_dram_parameter("w_shT", [k, dout_sh], F32, isOutput=False)
    wr_d = nc.declare_dram_parameter("w_red", [RED_ROWS, k], F32,
                                     isOutput=False)
    g_d = nc.declare_dram_parameter("gamma", [k], F32, isOutput=False)
    out_d = nc.declare_dram_parameter("out_sh", [tok_sh, dout_sh], F32,
                                      isOutput=True)

    with tile.TileContext(nc, num_cores=n_cores) as tc:
        with (
            tc.tile_pool(name="consts", bufs=1) as consts,
            tc.tile_pool(name="f32s", bufs=1) as f32s,
            tc.tile_pool(name="f16s", bufs=1) as f16s,
            tc.tile_pool(name="qt", bufs=1) as qtp,
            tc.tile_pool(name="outp", bufs=1) as outp,
            tc.tile_pool(name="psum", bufs=8, space="PSUM") as psum,
            tc.tile_pool(name="dram", bufs=1, space="DRAM") as dram,
        ):
            # ---- constants -------------------------------------------------
            cblock = consts.tile([128, 4 + n_rtile], F32)
            ones_col = cblock[:, 0:1]
            eps_t = cblock[:, 1:2]
            prev = cblock[:, 2:3]
            allv = cblock[:, 3:4]
            parts = cblock[:, 4:4 + n_rtile]
            nc.vector.memset(ones_col, 1.0)
            nc.vector.memset(eps_t, EPS)
            ones_row = consts.tile([1, 128], F32)
            nc.vector.memset(ones_row, 1.0)
            # gamma transposed to [p, t] with k = t*128 + p
            gT = consts.tile([128, kt], F32)
            g_ap = bass.AP(tensor=g_d, offset=0, ap=[[1, 128], [128, kt]])
            nc.sync.dma_start(out=gT, in_=g_ap)
            # sblock cols: 0 s_mean, 1 s_clip, 2 s_bc, 3 t_bc, 4 nt_bc
            sblock = consts.tile([128, 5], F32)
            s_bc = sblock[:, 2:3]
            t_bc = sblock[:, 3:4]
            nt_bc = sblock[:, 4:5]
            gs = consts.tile([128, kt], F32)     # gamma * s
            ngs = consts.tile([128, kt], F32)    # -gamma * s
            gs2 = consts.tile([128, kt], F32)    # gamma * s / 2
            invb = consts.tile([128, n_strip], F32)  # per-strip 1/rms

            # ---- phase S-pre: |w| partials, AllReduce input ----------------
            # Identical slices + reduce chain as the validated baseline so s
            # is bit-identical (the ternary pattern must not flip).
            wts = {}
            for i in range(n_rtile):
                wrt = f32s.tile([128, k], F32, tag="wt", bufs=8,
                                name=f"wr{i}")
                nc.gpsimd.dma_start(out=wrt,
                                    in_=wr_d[i * 128:(i + 1) * 128, :])
                nc.vector.tensor_reduce(
                    parts[:, i:i + 1], wrt, axis=mybir.AxisListType.X,
                    op=mybir.AluOpType.add, apply_absolute_value=True)
            nc.vector.tensor_reduce(prev, parts, axis=mybir.AxisListType.X,
                                    op=mybir.AluOpType.add)
            cc_in = dram.tile([128, 1], F32)
            cc_out = dram.tile([128, 1], F32, addr_space="Shared")
            nc.gpsimd.dma_start(out=cc_in, in_=prev)
            if use_cc:
                nc.gpsimd.collective_compute(
                    "AllReduce", mybir.AluOpType.add,
                    replica_groups=[list(range(n_cores))],
                    ins=[cc_in.opt()], outs=[cc_out.opt()],
                )
            else:
                nc.gpsimd.dma_start(out=cc_out, in_=cc_in)

            # ---- phase W-pre: f32 w k-tiles 0..7 resident before s ---------
            for i in range(kt - act_tiles):
                wts[i] = f32s.tile([128, dout_sh], F32, tag="wt", bufs=8,
                                   name=f"wq{i}")
                nc.gpsimd.dma_start(out=wts[i],
                                    in_=w_d[i * 128:(i + 1) * 128, :])

            # ---- strip preps (s-independent) -------------------------------
            xT_tiles = {}

            def prep_strip(j):
                xs = f16s.tile([128, k], F16, tag="xs", bufs=2, name=f"xs{j}")
                nc.gpsimd.dma_start(out=xs, in_=x_d[j * 128:(j + 1) * 128, :])
                xsq = f16s.tile([128, k], BF16, tag="xsq", bufs=1,
                                name=f"xsq{j}")
                sc = f16s.tile([128, 2], F32, tag="sc", bufs=3, name=f"sc{j}")
                ssq, rms = sc[:, 0:1], sc[:, 1:2]
                nc.scalar.activation(xsq, xs,
                                     mybir.ActivationFunctionType.Square,
                                     accum_out=ssq)
                nc.scalar.activation(rms, ssq,
                                     mybir.ActivationFunctionType.Sqrt,
                                     bias=eps_t, scale=1.0 / k)
                nc.vector.reciprocal(invb[:, j:j + 1], rms)
                xT = f16s.tile([128, kt, 128], F16, tag="xT", bufs=9,
                               name=f"xT{j}")
                nc.sync.dma_start_transpose(out=xT,
                                            in_=x_d[j * 128:(j + 1) * 128, :])
                xT_tiles[j] = xT

            for j in range(pre_strips):
                prep_strip(j)

            # ---- phase S-post: finish s after the AllReduce ----------------
            # allv on the gpsimd DMA queue: every gpsimd DMA emitted after it
            # stays quiet until the collective completes.
            nc.gpsimd.dma_start(out=allv, in_=cc_out)
            tot_ps = psum.tile([1, 1], F32, tag="mm")
            nc.tensor.matmul(tot_ps, lhsT=allv, rhs=ones_col,
                             start=True, stop=True)
            nc.scalar.activation(sblock[0:1, 0:1], tot_ps,
                                 mybir.ActivationFunctionType.Copy,
                                 scale=1.0 / W_COUNT)
            nc.vector.tensor_scalar_max(sblock[0:1, 1:2], sblock[0:1, 0:1],
                                        EPS)
            s_bc_ps = psum.tile([128, 1], F32, tag="mm")
            nc.tensor.matmul(s_bc_ps, lhsT=ones_row, rhs=sblock[0:1, 1:2],
                             start=True, stop=True)
            nc.scalar.copy(s_bc, s_bc_ps)
            nc.scalar.mul(t_bc, s_bc, 0.5)
            nc.scalar.mul(nt_bc, s_bc, -0.5)
            nc.vector.tensor_scalar(gs, gT, s_bc, None, mybir.AluOpType.mult)
            nc.vector.tensor_scalar(ngs, gs, -1.0, None, mybir.AluOpType.mult)
            nc.vector.tensor_scalar(gs2, gs, 0.5, None, mybir.AluOpType.mult)

            # ---- quantize --------------------------------------------------
            # qQ[q][p, u, o] = gamma*s*q(w[o, (kq*q+u)*128+p]) in fp16
            qQs = [qtp.tile([128, kq, dout_sh], F16, tag=f"qQ{q}",
                            name=f"qQ{q}") for q in range(n_kq)]

            def qslice(i):
                return qQs[i // kq][:, i % kq, :]

            for i in range(kt):
                if i not in wts:
                    wts[i] = f32s.tile([128, dout_sh], F32, tag="wt", bufs=8,
                                       name=f"wq{i}")
                    nc.gpsimd.dma_start(out=wts[i],
                                        in_=w_d[i * 128:(i + 1) * 128, :])
                wt = wts[i]
                if i < kt - act_tiles:
                    # DVE path: (w>t)*gs + (w<-t)*(-gs)
                    pos = f16s.tile([128, dout_sh], F16, tag="pos", bufs=2,
                                    name=f"pos{i}")
                    nc.vector.tensor_scalar(pos, wt, t_bc, gs[:, i:i + 1],
                                            mybir.AluOpType.is_gt,
                                            mybir.AluOpType.mult)
                    nm = f16s.tile([128, dout_sh], F16, tag="nm", bufs=2,
                                   name=f"nm{i}")
                    nc.vector.tensor_scalar(nm, wt, nt_bc, ngs[:, i:i + 1],
                                            mybir.AluOpType.is_lt,
                                            mybir.AluOpType.mult)
                    nc.vector.tensor_tensor(qslice(i), pos, nm,
                                            mybir.AluOpType.add)
                else:
                    # ACT path: (Sign(w-t) + Sign(w+t)) * gs/2
                    sp = f16s.tile([128, dout_sh], F16, tag="pos", bufs=2,
                                   name=f"sp{i}")
                    nc.scalar.activation(sp, wt,
                                         mybir.ActivationFunctionType.Sign,
                                         bias=nt_bc)
                    sn = f16s.tile([128, dout_sh], F16, tag="nm", bufs=2,
                                   name=f"sn{i}")
                    nc.scalar.activation(sn, wt,
                                         mybir.ActivationFunctionType.Sign,
                                         bias=t_bc)
                    tmp = f16s.tile([128, dout_sh], F16, tag="qtmp", bufs=1,
                                    name=f"qtmp{i}")
                    nc.vector.tensor_tensor(tmp, sp, sn, mybir.AluOpType.add)
                    nc.vector.tensor_scalar(qslice(i), tmp, gs2[:, i:i + 1],
                                            None, mybir.AluOpType.mult)

            # ---- main loop -------------------------------------------------
            def chain_chunk(j, d, ps, qtr):
                for u in range(kq):
                    t = qtr * kq + u
                    nc.tensor.matmul(
                        ps, lhsT=xT_tiles[j][:, t, :],
                        rhs=qQs[qtr][:, u, d * 512:(d + 1) * 512],
                        start=(t == 0), stop=(t == kt - 1))

            def finish_psum_tile(j, d, ps):
                ob = outp.tile([128, 512], F32, tag="ob", bufs=3,
                               name=f"ob{j}_{d}")
                nc.scalar.activation(
                    out=ob, in_=ps,
                    func=mybir.ActivationFunctionType.Copy,
                    scale=invb[:, j:j + 1])
                nc.scalar.dma_start(
                    out=out_d[j * 128:(j + 1) * 128, d * 512:(d + 1) * 512],
                    in_=ob)

            def emit_psum_tile(j, d):
                ps = psum.tile([128, 512], F32, tag="mm", name=f"ps{j}_{d}")
                for qtr in range(n_kq):
                    chain_chunk(j, d, ps, qtr)
                finish_psum_tile(j, d, ps)

            # block 0: quarter-interleaved across all 8 banks, 2 waves
            for wave in range(2):
                tiles = [(j, d) for d in (2 * wave, 2 * wave + 1)
                         for j in range(strip_blk)]
                pss = {(j, d): psum.tile([128, 512], F32, tag="mm",
                                         name=f"ps{j}_{d}")
                       for (j, d) in tiles}
                for qtr in range(n_kq):
                    for (j, d) in tiles:
                        chain_chunk(j, d, pss[(j, d)], qtr)
                for (j, d) in tiles:
                    finish_psum_tile(j, d, pss[(j, d)])

            # blocks 1..: plain deep-pipelined chains, preps interleaved
            next_prep = pre_strips
            for b in range(1, n_blk):
                for d in range(n_wtile // 4):
                    for j in range(b * strip_blk, (b + 1) * strip_blk):
                        emit_psum_tile(j, d)
                    # drip later strip-preps between psum groups
                    if next_prep < n_strip:
                        prep_strip(next_prep)
                        next_prep += 1

    nc.compile()
    return nc


_NC_CACHE = {}


def _get_nc():
    if "nc" not in _NC_CACHE:
        _NC_CACHE["nc"] = build_nc()
    return _NC_CACHE["nc"]


def make_in_maps(x, weight, gamma):
    """Shard + lay out host-side. x:[B,S,K] f32, weight:[DOUT,K] f32."""
    x = np.asarray(x, dtype=np.float32)
    weight = np.ascontiguousarray(np.asarray(weight, dtype=np.float32))
    gamma = np.ascontiguousarray(np.asarray(gamma, dtype=np.float32))

    x16 = x.reshape(B * S, K).astype(np.float16)
    wT = np.ascontiguousarray(weight.T)  # [K, DOUT] f32
    in_maps = []
    for c in range(N_CORES):
        rg, cg = c // CG, c % CG
        in_maps.append({
            "x_sh": np.ascontiguousarray(
                x16[rg * TOK_SH:(rg + 1) * TOK_SH]),
            "w_shT": np.ascontiguousarray(
                wT[:, cg * DOUT_SH:(cg + 1) * DOUT_SH]),
            "w_red": weight[c * RED_ROWS:(c + 1) * RED_ROWS],
            "gamma": gamma,
        })
    return in_maps


def kernel(x, weight, gamma):
    in_maps = make_in_maps(x, weight, gamma)
    nc = _get_nc()
    res = run_bass_kernel_spmd(nc, in_maps, list(range(N_CORES))).results

    out = np.empty((B * S, DOUT), dtype=np.float32)
    for c in range(N_CORES):
        rg, cg = c // CG, c % CG
        out[rg * TOK_SH:(rg + 1) * TOK_SH,
            cg * DOUT_SH:(cg + 1) * DOUT_SH] = res[c]["out_sh"]
    return out.reshape(B, S, DOUT)
